# revision 27
# baseline (speedup 1.0000x reference)
"""Bahdanau attention with coverage — Trainium2 Bass kernel.

Data-parallel over 8 NeuronCores: batch B=32 is sharded 4 per core; all
weights are replicated.  Per core the pipeline is:

  1. enc[b] is DMA-loaded HBM->SBUF with an in-DMA fp32->bf16 cast (SWDGE),
     kept resident in natural [l, e] layout for the context matmul.
  2. enc^T (contraction layout for the feature matmul) is built on-chip with
     xbar DMA transposes (bf16, 128x128 blocks).
  3. features^T[d, l] = W_h @ enc^T accumulates in PSUM (bf16 matmuls);
     the coverage term W_c[d]*cov[l] is folded in as a K=4 rank-1 matmul
     (batch-embedded lhsT) into the same accumulation group.
  4. The PSUM eviction is a single ScalarE pass: tanh(x + dec_b[d]) with the
     decoder-feature bias applied via the per-partition bias operand.
  5. scores = v . tanh(...) via M=4 matmuls whose lhsT embeds v in column b,
     so all 4 batches' scores land on 4 PSUM partitions -> softmax runs on
     4 lanes instead of 1.
  6. softmax without max-subtraction (|scores| <= sum|v| ~ 22.6, exp is safe
     in fp32); mask folded in as a multiply by mask (exp(-1e9) == 0 == x*0).
  7. context = attn @ enc via M=1 matmuls (attn column as lhsT, natural-layout
     bf16 enc as the moving operand).
"""

import numpy as np
import ml_dtypes

B, L, E, D = 32, 2048, 512, 512
NCORES = 8
BC = B // NCORES      # 4 batches per core
LC = L // 128         # 16 l-chunks of 128
LG = L // 512         # 4  l-groups of 512
ECH = E // 128        # 4  e-chunks
DCH = D // 128        # 4  d-chunks

_PROG = None          # cached Bass program
TRACE = False         # set by test.py to request an NTFF profile
LAST_RESULT = None    # BassKernelResults of the most recent run


def _build_program():
    import concourse.bass as bass
    import concourse.bacc as bacc
    import concourse.mybir as mybir
    from concourse import tile

    f32 = mybir.dt.float32
    bf16 = mybir.dt.bfloat16
    i32 = mybir.dt.int32
    AF = mybir.ActivationFunctionType
    ALU = mybir.AluOpType
    AX = mybir.AxisListType

    nc = bacc.Bacc("TRN2", target_bir_lowering=False, debug=False,
                   num_devices=NCORES)

    enc = nc.dram_tensor("enc", [BC, L, E], f32, kind="ExternalInput").ap()
    cov = nc.dram_tensor("cov", [BC, L], f32, kind="ExternalInput").ap()
    maski = nc.dram_tensor("maski", [BC, L], i32, kind="ExternalInput").ap()
    whT = nc.dram_tensor("whT", [128, ECH, D], bf16, kind="ExternalInput").ap()
    wsT = nc.dram_tensor("wsT", [128, DCH, D], bf16, kind="ExternalInput").ap()
    dsT = nc.dram_tensor("dsT", [128, DCH, BC], bf16, kind="ExternalInput").ap()
    ve = nc.dram_tensor("ve", [128, DCH, BC, BC], bf16, kind="ExternalInput").ap()
    wce = nc.dram_tensor("wce", [BC, DCH, BC, 128], bf16, kind="ExternalInput").ap()
    bsr = nc.dram_tensor("bsr", [1, DCH, 128], bf16, kind="ExternalInput").ap()
    id4 = nc.dram_tensor("id4", [BC, BC], f32, kind="ExternalInput").ap()
    ctx_out = nc.dram_tensor("ctx_out", [BC, E], f32, kind="ExternalOutput").ap()
    attn_out = nc.dram_tensor("attn_out", [BC, L], f32, kind="ExternalOutput").ap()

    with tile.TileContext(nc) as tc:
        with tc.tile_pool(name="const", bufs=1) as cpool, \
             tc.tile_pool(name="scorep", bufs=1, space="PSUM") as spool:

            # ---- persistent SBUF state -------------------------------------
            whT_sb = cpool.tile([128, ECH, D], bf16)
            wsT_sb = cpool.tile([128, DCH, D], bf16)
            dsT_sb = cpool.tile([128, DCH, BC], bf16)
            ve_sb = cpool.tile([128, DCH, BC, BC], bf16)
            wce_sb = cpool.tile([BC, DCH, BC, 128], bf16)
            bsr_sb = cpool.tile([1, DCH, 128], bf16)
            ones_sb = cpool.tile([1, BC], bf16)
            covf_sb = cpool.tile([BC, L], f32)
            covb_sb = cpool.tile([BC, L], bf16)
            maski_sb = cpool.tile([BC, L], i32)
            maskf_sb = cpool.tile([BC, L], f32)
            dec_sb = cpool.tile([128, DCH, BC], f32)
            id4_sb = cpool.tile([BC, BC], f32)
            attnT_sb = cpool.tile([128, LC, BC], bf16)

            nc.sync.dma_start(whT_sb[:], whT)
            nc.sync.dma_start(wsT_sb[:], wsT)
            nc.sync.dma_start(dsT_sb[:], dsT)
            nc.sync.dma_start(ve_sb[:], ve)
            nc.sync.dma_start(wce_sb[:], wce)
            nc.sync.dma_start(bsr_sb[:], bsr)
            nc.sync.dma_start(covf_sb[:], cov)
            nc.sync.dma_start(maski_sb[:], maski)
            nc.sync.dma_start(id4_sb[:], id4)
            nc.vector.tensor_copy(covb_sb[:], covf_sb[:])
            nc.vector.tensor_copy(maskf_sb[:], maski_sb[:])
            nc.vector.memset(ones_sb[:], 1.0)

            # scores PSUM, one bank per l-group, accumulated across the whole
            # batch loop (16 matmuls each: 4 batches x 4 d-chunks)
            psumS = [spool.tile([BC, 512], f32, name=f"psumS{g}", tag=f"psumS{g}")
                     for g in range(LG)]

            # ---- decoder features: dec^T[d, b] = W_s @ dec_state^T + b_s ---
            with tc.tile_pool(name="decp", bufs=2, space="PSUM") as dpool:
                for dc in range(DCH):
                    pd = dpool.tile([128, BC], f32, name=f"pd{dc}", tag="pd")
                    for kc in range(DCH):
                        nc.tensor.matmul(
                            pd[:], wsT_sb[:, kc, dc * 128:(dc + 1) * 128],
                            dsT_sb[:, kc, :],
                            start=(kc == 0), stop=False)
                    nc.tensor.matmul(
                        pd[:], bsr_sb[:, dc, :], ones_sb[:],
                        start=False, stop=True)
                    nc.vector.tensor_copy(dec_sb[:, dc, :], pd[:])

            # natural-layout bf16 enc, resident for the whole kernel
            enc_bf = [cpool.tile([128, LC, E], bf16, name=f"encbf{b}",
                                 tag=f"encbf{b}")
                      for b in range(BC)]

            with tc.tile_pool(name="tpose", bufs=2) as tpool, \
                 tc.tile_pool(name="featp", bufs=2, space="PSUM") as fpool, \
                 tc.tile_pool(name="tanhp", bufs=3) as hpool, \
                 tc.tile_pool(name="dramp", bufs=1, space="DRAM") as dpool:
                encs = dpool.tile([BC, L, E], bf16)
                for b in range(BC):
                    encT = tpool.tile([128, ECH, L], bf16, name="encT",
                                      tag="encT")
                    # l-half chunked pipeline: cast-load (SWDGE, fp32->bf16),
                    # write bf16 back to DRAM, then big contiguous xbar
                    # transpose loads ([1024,128]->[128,1024]); halves let the
                    # first feature matmuls start after ~1/8 of the load.
                    for h2 in range(2):
                        lr = slice(h2 * (L // 2), (h2 + 1) * (L // 2))
                        cr8 = slice(h2 * (LC // 2), (h2 + 1) * (LC // 2))
                        nc.gpsimd.dma_start(
                            enc_bf[b][:, cr8, :],
                            enc[b, lr].rearrange("(lc p) e -> p lc e", p=128))
                        nc.scalar.dma_start(
                            encs[b, lr].rearrange("(lc p) e -> p lc e", p=128),
                            enc_bf[b][:, cr8, :])
                        for ec in range(ECH):
                            nc.sync.dma_start(
                                encT[:, ec, lr],
                                encs[b][lr, ec * 128:(ec + 1) * 128],
                                transpose=True)

                    for dc in range(DCH):
                        for h in range(2):
                            pf = fpool.tile([128, 1024], f32, name="pf",
                                            tag="pf")
                            for ec in range(ECH):
                                for j in range(2):
                                    lg = 2 * h + j
                                    nc.tensor.matmul(
                                        pf[:, j * 512:(j + 1) * 512],
                                        whT_sb[:, ec, dc * 128:(dc + 1) * 128],
                                        encT[:, ec, lg * 512:(lg + 1) * 512],
                                        start=(ec == 0), stop=False)
                            for j in range(2):
                                lg = 2 * h + j
                                nc.tensor.matmul(
                                    pf[:, j * 512:(j + 1) * 512],
                                    wce_sb[:, dc, b, :],
                                    covb_sb[:, lg * 512:(lg + 1) * 512],
                                    start=False, stop=True)
                            th = hpool.tile([128, 1024], bf16, name="th",
                                            tag="th")
                            nc.scalar.activation(
                                th[:], pf[:], AF.Tanh,
                                bias=dec_sb[:, dc, b:b + 1])
                            for j in range(2):
                                lg = 2 * h + j
                                nc.tensor.matmul(
                                    psumS[lg][:],
                                    ve_sb[:, dc, b, :],
                                    th[:, j * 512:(j + 1) * 512],
                                    start=(b == 0 and dc == 0),
                                    stop=(b == BC - 1 and dc == DCH - 1),
                                    skip_group_check=True)

            # ---- softmax over l (4 batches on 4 partition lanes) -----------
            expS = cpool.tile([BC, L], f32)
            attnU = cpool.tile([BC, L], f32)
            sums = cpool.tile([BC, 1], f32)
            rsum = cpool.tile([BC, 1], f32)
            attnN = cpool.tile([BC, L], f32)
            for g in range(LG):
                nc.scalar.activation(expS[:, g * 512:(g + 1) * 512],
                                     psumS[g][:], AF.Exp)
            nc.vector.tensor_tensor(attnU[:], expS[:], maskf_sb[:], ALU.mult)
            nc.vector.reduce_sum(sums[:], attnU[:], axis=AX.X)
            nc.vector.reciprocal(rsum[:], sums[:])
            nc.vector.tensor_scalar(attnN[:], attnU[:], rsum[:, 0:1], None,
                                    ALU.mult)
            nc.sync.dma_start(attn_out, attnN[:])

            with tc.tile_pool(name="ctxp", bufs=1, space="PSUM") as xpool, \
                 tc.tile_pool(name="ctxsb", bufs=1) as xsb:
                # attn^T via PE transposes: [4, 128] slices -> [128, 4] PSUM
                attnT_ps = xpool.tile([128, LC, BC], f32)
                for lc in range(LC):
                    nc.tensor.transpose(attnT_ps[:, lc, :],
                                        attnN[:, lc * 128:(lc + 1) * 128],
                                        id4_sb[:])
                nc.vector.tensor_copy(attnT_sb[:], attnT_ps[:])

                # ---- context = attn @ enc ----------------------------------
                for b in range(BC):
                    pc = xpool.tile([1, E], f32, name="pc", tag="pc", bufs=2)
                    for lc in range(LC):
                        nc.tensor.matmul(
                            pc[:], attnT_sb[:, lc, b:b + 1],
                            enc_bf[b][:, lc, :],
                            start=(lc == 0), stop=(lc == LC - 1))
                    cr = xsb.tile([1, E], f32, name="cr", tag="cr", bufs=2)
                    nc.vector.tensor_copy(cr[:], pc[:])
                    nc.sync.dma_start(ctx_out[b:b + 1, :], cr[:])

    nc.compile()
    return nc


def _prep_inputs(enc_outputs, dec_state, coverage, mask, W_h, W_s, b_s, W_c, v):
    """Host-side prep: per-core sharding + (tiny) weight relayouts."""
    bf16 = ml_dtypes.bfloat16
    enc_outputs = np.ascontiguousarray(enc_outputs, dtype=np.float32)
    coverage = np.ascontiguousarray(coverage, dtype=np.float32)
    mask = np.ascontiguousarray(mask, dtype=np.int32)

    # W_h^T [e, d] -> [p, ec, d]
    whT = np.ascontiguousarray(
        W_h.T.astype(bf16).reshape(ECH, 128, D).transpose(1, 0, 2))
    # W_s^T [k, d] -> [p, kc, d]
    wsT = np.ascontiguousarray(
        W_s.T.astype(bf16).reshape(DCH, 128, D).transpose(1, 0, 2))
    # b_s -> [1, dc, p] (folded into the dec matmul via a ones rhs)
    bsr = np.ascontiguousarray(b_s.astype(bf16).reshape(1, DCH, 128))
    id4 = np.eye(BC, dtype=np.float32)
    # v embedded per (dc, target batch-column)
    ve = np.zeros((128, DCH, BC, BC), dtype=bf16)
    vb = v[0].astype(bf16).reshape(DCH, 128)
    for dc in range(DCH):
        for b in range(BC):
            ve[:, dc, b, b] = vb[dc]
    # W_c embedded per (dc, batch-row)
    wce = np.zeros((BC, DCH, BC, 128), dtype=bf16)
    wcb = W_c[:, 0].astype(bf16).reshape(DCH, 128)
    for dc in range(DCH):
        for b in range(BC):
            wce[b, dc, b, :] = wcb[dc]

    in_maps = []
    for c in range(NCORES):
        sl = slice(c * BC, (c + 1) * BC)
        dsT = np.ascontiguousarray(
            dec_state[sl].T.astype(bf16).reshape(DCH, 128, BC)
            .transpose(1, 0, 2))
        in_maps.append({
            "enc": enc_outputs[sl],
            "cov": coverage[sl],
            "maski": mask[sl],
            "whT": whT, "wsT": wsT, "dsT": dsT, "ve": ve, "wce": wce,
            "bsr": bsr, "id4": id4,
        })
    return in_maps


def kernel(enc_outputs, dec_state, coverage, mask, W_h, W_s, b_s, W_c, v):
    global _PROG, LAST_RESULT
    from concourse.bass_utils import run_bass_kernel_spmd

    enc_outputs = np.asarray(enc_outputs)
    dec_state = np.asarray(dec_state)
    coverage = np.asarray(coverage)
    mask = np.asarray(mask)
    W_h = np.asarray(W_h)
    W_s = np.asarray(W_s)
    b_s = np.asarray(b_s)
    W_c = np.asarray(W_c)
    v = np.asarray(v)

    if _PROG is None:
        _PROG = _build_program()
    nc = _PROG

    in_maps = _prep_inputs(enc_outputs, dec_state, coverage, mask,
                           W_h, W_s, b_s, W_c, v)
    res = run_bass_kernel_spmd(nc, in_maps, core_ids=list(range(NCORES)),
                               trace=TRACE)
    LAST_RESULT = res

    context = np.empty((B, E), dtype=np.float32)
    attn = np.empty((B, L), dtype=np.float32)
    for c in range(NCORES):
        sl = slice(c * BC, (c + 1) * BC)
        context[sl] = res.results[c]["ctx_out"]
        attn[sl] = res.results[c]["attn_out"]
    return context, attn


# revision 33
# speedup vs baseline: 1.4502x; 1.4502x over previous
"""Bahdanau attention with coverage — Trainium2 Bass kernel.

Data-parallel over 8 NeuronCores: batch B=32 is sharded 4 per core; all
weights are replicated.  Per core the pipeline is:

  1. enc[b] is DMA-loaded HBM->SBUF with an in-DMA fp32->bf16 cast (SWDGE),
     kept resident in natural [l, e] layout for the context matmul.
  2. enc^T (contraction layout for the feature matmul) is built on-chip with
     xbar DMA transposes (bf16, 128x128 blocks).
  3. features^T[d, l] = W_h @ enc^T accumulates in PSUM (bf16 matmuls);
     the coverage term W_c[d]*cov[l] is folded in as a K=4 rank-1 matmul
     (batch-embedded lhsT) into the same accumulation group.
  4. The PSUM eviction is a single ScalarE pass: tanh(x + dec_b[d]) with the
     decoder-feature bias applied via the per-partition bias operand.
  5. scores = v . tanh(...) via M=4 matmuls whose lhsT embeds v in column b,
     so all 4 batches' scores land on 4 PSUM partitions -> softmax runs on
     4 lanes instead of 1.
  6. softmax without max-subtraction (|scores| <= sum|v| ~ 22.6, exp is safe
     in fp32); mask folded in as a multiply by mask (exp(-1e9) == 0 == x*0).
  7. context = attn @ enc via M=1 matmuls (attn column as lhsT, natural-layout
     bf16 enc as the moving operand).
"""

import numpy as np
import ml_dtypes

B, L, E, D = 32, 2048, 512, 512
NCORES = 8
BC = B // NCORES      # 4 batches per core
LC = L // 128         # 16 l-chunks of 128
LG = L // 512         # 4  l-groups of 512
ECH = E // 128        # 4  e-chunks
DCH = D // 128        # 4  d-chunks

_PROG = None          # cached Bass program
TRACE = False         # set by test.py to request an NTFF profile
LAST_RESULT = None    # BassKernelResults of the most recent run


def _build_program():
    import concourse.bass as bass
    import concourse.bacc as bacc
    import concourse.mybir as mybir
    from concourse import tile

    f32 = mybir.dt.float32
    bf16 = mybir.dt.bfloat16
    i32 = mybir.dt.int32
    AF = mybir.ActivationFunctionType
    ALU = mybir.AluOpType
    AX = mybir.AxisListType

    nc = bacc.Bacc("TRN2", target_bir_lowering=False, debug=False,
                   num_devices=NCORES)

    enc = nc.dram_tensor("enc", [BC, L, E], f32, kind="ExternalInput").ap()
    cov = nc.dram_tensor("cov", [BC, L], f32, kind="ExternalInput").ap()
    maski = nc.dram_tensor("maski", [BC, L], i32, kind="ExternalInput").ap()
    whT = nc.dram_tensor("whT", [128, ECH, D], bf16, kind="ExternalInput").ap()
    wsT = nc.dram_tensor("wsT", [128, DCH, D], bf16, kind="ExternalInput").ap()
    dsT = nc.dram_tensor("dsT", [128, DCH, BC], bf16, kind="ExternalInput").ap()
    ve = nc.dram_tensor("ve", [128, DCH, BC, BC], bf16, kind="ExternalInput").ap()
    wce = nc.dram_tensor("wce", [BC, DCH, BC, 128], bf16, kind="ExternalInput").ap()
    bsr = nc.dram_tensor("bsr", [1, DCH, 128], bf16, kind="ExternalInput").ap()
    id4 = nc.dram_tensor("id4", [BC, BC], f32, kind="ExternalInput").ap()
    idT = nc.dram_tensor("idT", [128, 128], bf16, kind="ExternalInput").ap()
    ctx_out = nc.dram_tensor("ctx_out", [BC, E], f32, kind="ExternalOutput").ap()
    attn_out = nc.dram_tensor("attn_out", [BC, L], f32, kind="ExternalOutput").ap()

    with tile.TileContext(nc) as tc:
        with tc.tile_pool(name="const", bufs=1) as cpool, \
             tc.tile_pool(name="scorep", bufs=1, space="PSUM") as spool:

            # ---- persistent SBUF state -------------------------------------
            whT_sb = cpool.tile([128, ECH, D], bf16)
            wsT_sb = cpool.tile([128, DCH, D], bf16)
            dsT_sb = cpool.tile([128, DCH, BC], bf16)
            ve_sb = cpool.tile([128, DCH, BC, BC], bf16)
            wce_sb = cpool.tile([BC, DCH, BC, 128], bf16)
            bsr_sb = cpool.tile([1, DCH, 128], bf16)
            ones_sb = cpool.tile([1, BC], bf16)
            covf_sb = cpool.tile([BC, L], f32)
            covb_sb = cpool.tile([BC, L], bf16)
            maski_sb = cpool.tile([BC, L], i32)
            maskf_sb = cpool.tile([BC, L], f32)
            dec_sb = cpool.tile([128, DCH, BC], f32)
            id4_sb = cpool.tile([BC, BC], f32)
            idT_sb = cpool.tile([128, 128], bf16)
            attnT_sb = cpool.tile([128, LC, BC], bf16)

            nc.sync.dma_start(whT_sb[:], whT)
            nc.sync.dma_start(wsT_sb[:], wsT)
            nc.sync.dma_start(dsT_sb[:], dsT)
            nc.sync.dma_start(ve_sb[:], ve)
            nc.sync.dma_start(wce_sb[:], wce)
            nc.sync.dma_start(bsr_sb[:], bsr)
            nc.sync.dma_start(covf_sb[:], cov)
            nc.sync.dma_start(maski_sb[:], maski)
            nc.sync.dma_start(id4_sb[:], id4)
            nc.sync.dma_start(idT_sb[:], idT)
            nc.vector.tensor_copy(covb_sb[:], covf_sb[:])
            nc.vector.tensor_copy(maskf_sb[:], maski_sb[:])
            nc.vector.memset(ones_sb[:], 1.0)

            # scores PSUM, one bank per l-group, accumulated across the whole
            # batch loop (16 matmuls each: 4 batches x 4 d-chunks)
            psumS = [spool.tile([BC, 512], f32, name=f"psumS{g}", tag=f"psumS{g}")
                     for g in range(LG)]

            # ---- decoder features: dec^T[d, b] = W_s @ dec_state^T + b_s ---
            with tc.tile_pool(name="decp", bufs=2, space="PSUM") as dpool:
                for dc in range(DCH):
                    pd = dpool.tile([128, BC], f32, name=f"pd{dc}", tag="pd")
                    for kc in range(DCH):
                        nc.tensor.matmul(
                            pd[:], wsT_sb[:, kc, dc * 128:(dc + 1) * 128],
                            dsT_sb[:, kc, :],
                            start=(kc == 0), stop=False)
                    nc.tensor.matmul(
                        pd[:], bsr_sb[:, dc, :], ones_sb[:],
                        start=False, stop=True)
                    nc.vector.tensor_copy(dec_sb[:, dc, :], pd[:])

            # natural-layout bf16 enc, resident for the whole kernel
            enc_bf = [cpool.tile([128, LC, E], bf16, name=f"encbf{b}",
                                 tag=f"encbf{b}")
                      for b in range(BC)]

            with tc.tile_pool(name="tpose", bufs=2) as tpool, \
                 tc.tile_pool(name="tposp", bufs=2, space="PSUM") as tpsp, \
                 tc.tile_pool(name="featp", bufs=2, space="PSUM") as fpool, \
                 tc.tile_pool(name="tanhp", bufs=3) as hpool:
                for b in range(BC):
                    # fp32 -> bf16 cast inside the DMA (SWDGE); quarter
                    # chunks so the first PE transposes can start early
                    for q in range(4):
                        cq = slice(q * (LC // 4), (q + 1) * (LC // 4))
                        lr = slice(q * (L // 4), (q + 1) * (L // 4))
                        nc.gpsimd.dma_start(
                            enc_bf[b][:, cq, :],
                            enc[b, lr].rearrange("(lc p) e -> p lc e", p=128))

                    # enc^T via PE transposes (LDWEIGHTS-bound, ~110ns/tile),
                    # evicted PSUM->SBUF by DVE in [128, 512] chunks
                    encT = tpool.tile([128, ECH, L], bf16, name="encT",
                                      tag="encT")
                    for lg in range(LG):
                        for ec in range(ECH):
                            pt = tpsp.tile([128, 4, 128], bf16, name="pt",
                                           tag="pt")
                            for j in range(4):
                                lc = lg * 4 + j
                                nc.tensor.transpose(
                                    pt[:, j, :],
                                    enc_bf[b][:, lc, ec * 128:(ec + 1) * 128],
                                    idT_sb[:])
                            nc.vector.tensor_copy(
                                encT[:, ec, lg * 512:(lg + 1) * 512], pt[:])

                    for dc in range(DCH):
                        for lg in range(LG):
                            pf = fpool.tile([128, 512], f32, name="pf",
                                            tag="pf")
                            for ec in range(ECH):
                                nc.tensor.matmul(
                                    pf[:],
                                    whT_sb[:, ec, dc * 128:(dc + 1) * 128],
                                    encT[:, ec, lg * 512:(lg + 1) * 512],
                                    start=(ec == 0), stop=False)
                            nc.tensor.matmul(
                                pf[:],
                                wce_sb[:, dc, b, :],
                                covb_sb[:, lg * 512:(lg + 1) * 512],
                                start=False, stop=True)
                            th = hpool.tile([128, 512], bf16, name="th",
                                            tag="th")
                            nc.scalar.activation(
                                th[:], pf[:], AF.Tanh,
                                bias=dec_sb[:, dc, b:b + 1])
                            nc.tensor.matmul(
                                psumS[lg][:],
                                ve_sb[:, dc, b, :],
                                th[:],
                                start=(b == 0 and dc == 0),
                                stop=(b == BC - 1 and dc == DCH - 1),
                                skip_group_check=True)

            # ---- softmax over l (4 batches on 4 partition lanes) -----------
            expS = cpool.tile([BC, L], f32)
            attnU = cpool.tile([BC, L], f32)
            sums = cpool.tile([BC, 1], f32)
            rsum = cpool.tile([BC, 1], f32)
            attnN = cpool.tile([BC, L], f32)
            for g in range(LG):
                nc.scalar.activation(expS[:, g * 512:(g + 1) * 512],
                                     psumS[g][:], AF.Exp)
            nc.vector.tensor_tensor(attnU[:], expS[:], maskf_sb[:], ALU.mult)
            nc.vector.reduce_sum(sums[:], attnU[:], axis=AX.X)
            nc.vector.reciprocal(rsum[:], sums[:])
            nc.vector.tensor_scalar(attnN[:], attnU[:], rsum[:, 0:1], None,
                                    ALU.mult)
            nc.sync.dma_start(attn_out, attnN[:])

            with tc.tile_pool(name="ctxp", bufs=1, space="PSUM") as xpool, \
                 tc.tile_pool(name="ctxsb", bufs=1) as xsb:
                # attn^T via PE transposes: [4, 128] slices -> [128, 4] PSUM
                attnT_ps = xpool.tile([128, LC, BC], f32)
                for lc in range(LC):
                    nc.tensor.transpose(attnT_ps[:, lc, :],
                                        attnN[:, lc * 128:(lc + 1) * 128],
                                        id4_sb[:])
                nc.vector.tensor_copy(attnT_sb[:], attnT_ps[:])

                # ---- context = attn @ enc ----------------------------------
                for b in range(BC):
                    pc = xpool.tile([1, E], f32, name="pc", tag="pc", bufs=2)
                    for lc in range(LC):
                        nc.tensor.matmul(
                            pc[:], attnT_sb[:, lc, b:b + 1],
                            enc_bf[b][:, lc, :],
                            start=(lc == 0), stop=(lc == LC - 1))
                    cr = xsb.tile([1, E], f32, name="cr", tag="cr", bufs=2)
                    nc.vector.tensor_copy(cr[:], pc[:])
                    nc.sync.dma_start(ctx_out[b:b + 1, :], cr[:])

    nc.compile()
    return nc


def _prep_inputs(enc_outputs, dec_state, coverage, mask, W_h, W_s, b_s, W_c, v):
    """Host-side prep: per-core sharding + (tiny) weight relayouts."""
    bf16 = ml_dtypes.bfloat16
    enc_outputs = np.ascontiguousarray(enc_outputs, dtype=np.float32)
    coverage = np.ascontiguousarray(coverage, dtype=np.float32)
    mask = np.ascontiguousarray(mask, dtype=np.int32)

    # W_h^T [e, d] -> [p, ec, d]
    whT = np.ascontiguousarray(
        W_h.T.astype(bf16).reshape(ECH, 128, D).transpose(1, 0, 2))
    # W_s^T [k, d] -> [p, kc, d]
    wsT = np.ascontiguousarray(
        W_s.T.astype(bf16).reshape(DCH, 128, D).transpose(1, 0, 2))
    # b_s -> [1, dc, p] (folded into the dec matmul via a ones rhs)
    bsr = np.ascontiguousarray(b_s.astype(bf16).reshape(1, DCH, 128))
    id4 = np.eye(BC, dtype=np.float32)
    idT = np.eye(128, dtype=bf16)
    # v embedded per (dc, target batch-column)
    ve = np.zeros((128, DCH, BC, BC), dtype=bf16)
    vb = v[0].astype(bf16).reshape(DCH, 128)
    for dc in range(DCH):
        for b in range(BC):
            ve[:, dc, b, b] = vb[dc]
    # W_c embedded per (dc, batch-row)
    wce = np.zeros((BC, DCH, BC, 128), dtype=bf16)
    wcb = W_c[:, 0].astype(bf16).reshape(DCH, 128)
    for dc in range(DCH):
        for b in range(BC):
            wce[b, dc, b, :] = wcb[dc]

    in_maps = []
    for c in range(NCORES):
        sl = slice(c * BC, (c + 1) * BC)
        dsT = np.ascontiguousarray(
            dec_state[sl].T.astype(bf16).reshape(DCH, 128, BC)
            .transpose(1, 0, 2))
        in_maps.append({
            "enc": enc_outputs[sl],
            "cov": coverage[sl],
            "maski": mask[sl],
            "whT": whT, "wsT": wsT, "dsT": dsT, "ve": ve, "wce": wce,
            "bsr": bsr, "id4": id4, "idT": idT,
        })
    return in_maps


def kernel(enc_outputs, dec_state, coverage, mask, W_h, W_s, b_s, W_c, v):
    global _PROG, LAST_RESULT
    from concourse.bass_utils import run_bass_kernel_spmd

    enc_outputs = np.asarray(enc_outputs)
    dec_state = np.asarray(dec_state)
    coverage = np.asarray(coverage)
    mask = np.asarray(mask)
    W_h = np.asarray(W_h)
    W_s = np.asarray(W_s)
    b_s = np.asarray(b_s)
    W_c = np.asarray(W_c)
    v = np.asarray(v)

    if _PROG is None:
        _PROG = _build_program()
    nc = _PROG

    in_maps = _prep_inputs(enc_outputs, dec_state, coverage, mask,
                           W_h, W_s, b_s, W_c, v)
    res = run_bass_kernel_spmd(nc, in_maps, core_ids=list(range(NCORES)),
                               trace=TRACE)
    LAST_RESULT = res

    context = np.empty((B, E), dtype=np.float32)
    attn = np.empty((B, L), dtype=np.float32)
    for c in range(NCORES):
        sl = slice(c * BC, (c + 1) * BC)
        context[sl] = res.results[c]["ctx_out"]
        attn[sl] = res.results[c]["attn_out"]
    return context, attn


# revision 46
# speedup vs baseline: 1.4723x; 1.0152x over previous
"""Bahdanau attention with coverage — Trainium2 Bass kernel.

Data-parallel over 8 NeuronCores: batch B=32 is sharded 4 per core; all
weights are replicated.  Per core the pipeline is:

  1. enc[b] is DMA-loaded HBM->SBUF with an in-DMA fp32->bf16 cast (SWDGE),
     kept resident in natural [l, e] layout for the context matmul.
  2. enc^T (contraction layout for the feature matmul) is built on-chip with
     xbar DMA transposes (bf16, 128x128 blocks).
  3. features^T[d, l] = W_h @ enc^T accumulates in PSUM (bf16 matmuls);
     the coverage term W_c[d]*cov[l] is folded in as a K=4 rank-1 matmul
     (batch-embedded lhsT) into the same accumulation group.
  4. The PSUM eviction is a single ScalarE pass: tanh(x + dec_b[d]) with the
     decoder-feature bias applied via the per-partition bias operand.
  5. scores = v . tanh(...) via M=4 matmuls whose lhsT embeds v in column b,
     so all 4 batches' scores land on 4 PSUM partitions -> softmax runs on
     4 lanes instead of 1.
  6. softmax without max-subtraction (|scores| <= sum|v| ~ 22.6, exp is safe
     in fp32); mask folded in as a multiply by mask (exp(-1e9) == 0 == x*0).
  7. context = attn @ enc via M=1 matmuls (attn column as lhsT, natural-layout
     bf16 enc as the moving operand).
"""

import numpy as np
import ml_dtypes

B, L, E, D = 32, 2048, 512, 512
NCORES = 8
BC = B // NCORES      # 4 batches per core
LC = L // 128         # 16 l-chunks of 128
LG = L // 512         # 4  l-groups of 512
ECH = E // 128        # 4  e-chunks
DCH = D // 128        # 4  d-chunks

_PROG = None          # cached Bass program
TRACE = False         # set by test.py to request an NTFF profile
LAST_RESULT = None    # BassKernelResults of the most recent run
FP8 = False           # fp8-e4m3 DoubleRow feature matmuls (else bf16)
W_SCALE = 64.0        # host pre-scale of W_h/W_c keeping fp8 operands normal


def _build_program():
    import concourse.bass as bass
    import concourse.bacc as bacc
    import concourse.mybir as mybir
    from concourse import tile
    from concourse.tile import add_dep_helper

    f32 = mybir.dt.float32
    bf16 = mybir.dt.bfloat16
    fp8 = mybir.dt.float8e4
    i32 = mybir.dt.int32
    AF = mybir.ActivationFunctionType
    ALU = mybir.AluOpType
    AX = mybir.AxisListType
    DR = mybir.MatmulPerfMode.DoubleRow

    nc = bacc.Bacc("TRN2", target_bir_lowering=False, debug=False,
                   num_devices=NCORES)

    enc = nc.dram_tensor("enc", [BC, L, E], f32, kind="ExternalInput").ap()
    cov = nc.dram_tensor("cov", [BC, L], f32, kind="ExternalInput").ap()
    maski = nc.dram_tensor("maski", [BC, L], i32, kind="ExternalInput").ap()
    if FP8:
        # [ki, drc, i, dc, m] with e = drc*256 + i*128 + ki, pre-scaled
        whT = nc.dram_tensor("whT", [128, 2, 2, DCH, 128], fp8,
                             kind="ExternalInput").ap()
    else:
        whT = nc.dram_tensor("whT", [128, ECH, D], bf16,
                             kind="ExternalInput").ap()
    wsT = nc.dram_tensor("wsT", [128, DCH, D], bf16, kind="ExternalInput").ap()
    dsT = nc.dram_tensor("dsT", [128, DCH, BC], bf16, kind="ExternalInput").ap()
    ve = nc.dram_tensor("ve", [128, DCH, BC, BC], bf16, kind="ExternalInput").ap()
    wce = nc.dram_tensor("wce", [BC, DCH, BC, 128], bf16, kind="ExternalInput").ap()
    bsr = nc.dram_tensor("bsr", [1, DCH, 128], bf16, kind="ExternalInput").ap()
    id4 = nc.dram_tensor("id4", [BC, BC], f32, kind="ExternalInput").ap()
    idT = nc.dram_tensor("idT", [128, 128], bf16, kind="ExternalInput").ap()
    ctx_out = nc.dram_tensor("ctx_out", [BC, E], f32, kind="ExternalOutput").ap()
    attn_out = nc.dram_tensor("attn_out", [BC, L], f32, kind="ExternalOutput").ap()

    with tile.TileContext(nc) as tc:
        with tc.tile_pool(name="const", bufs=1) as cpool, \
             tc.tile_pool(name="scorep", bufs=1, space="PSUM") as spool:

            # ---- persistent SBUF state -------------------------------------
            if FP8:
                whT_sb = cpool.tile([128, 2, 2, DCH, 128], fp8)
            else:
                whT_sb = cpool.tile([128, ECH, D], bf16)
            wsT_sb = cpool.tile([128, DCH, D], bf16)
            dsT_sb = cpool.tile([128, DCH, BC], bf16)
            ve_sb = cpool.tile([128, DCH, BC, BC], bf16)
            wce_sb = cpool.tile([BC, DCH, BC, 128], bf16)
            bsr_sb = cpool.tile([1, DCH, 128], bf16)
            ones_sb = cpool.tile([1, BC], bf16)
            covf_sb = cpool.tile([BC, L], f32)
            covb_sb = cpool.tile([BC, L], bf16)
            maski_sb = cpool.tile([BC, L], i32)
            maskf_sb = cpool.tile([BC, L], f32)
            dec_sb = cpool.tile([128, DCH, BC], f32)
            id4_sb = cpool.tile([BC, BC], f32)
            idT_sb = cpool.tile([128, 128], bf16)
            attnT_sb = cpool.tile([128, LC, BC], bf16)

            nc.sync.dma_start(whT_sb[:], whT)
            nc.sync.dma_start(wsT_sb[:], wsT)
            nc.sync.dma_start(dsT_sb[:], dsT)
            nc.sync.dma_start(ve_sb[:], ve)
            nc.sync.dma_start(wce_sb[:], wce)
            nc.sync.dma_start(bsr_sb[:], bsr)
            nc.sync.dma_start(covf_sb[:], cov)
            nc.sync.dma_start(maski_sb[:], maski)
            nc.sync.dma_start(id4_sb[:], id4)
            nc.sync.dma_start(idT_sb[:], idT)
            nc.vector.tensor_copy(covb_sb[:], covf_sb[:])
            nc.vector.tensor_copy(maskf_sb[:], maski_sb[:])
            nc.vector.memset(ones_sb[:], 1.0)

            # scores PSUM: one 4-bank tile, rows = batches (the scores lhsT
            # embeds v in column b), accumulated across the whole batch loop
            psumS = spool.tile([BC, LG, 512], f32)

            # ---- decoder features: dec^T[d, b] = W_s @ dec_state^T + b_s ---
            with tc.tile_pool(name="decp", bufs=2, space="PSUM") as dpool:
                for dc in range(DCH):
                    pd = dpool.tile([128, BC], f32, name=f"pd{dc}", tag="pd")
                    for kc in range(DCH):
                        nc.tensor.matmul(
                            pd[:], wsT_sb[:, kc, dc * 128:(dc + 1) * 128],
                            dsT_sb[:, kc, :],
                            start=(kc == 0), stop=False)
                    nc.tensor.matmul(
                        pd[:], bsr_sb[:, dc, :], ones_sb[:],
                        start=False, stop=True)
                    nc.vector.tensor_copy(dec_sb[:, dc, :], pd[:])

            # natural-layout bf16 enc, resident for the whole kernel
            enc_bf = [cpool.tile([128, LC, E], bf16, name=f"encbf{b}",
                                 tag=f"encbf{b}")
                      for b in range(BC)]

            with tc.tile_pool(name="tpose", bufs=2) as tpool, \
                 tc.tile_pool(name="tposp", bufs=2, space="PSUM") as tpsp, \
                 tc.tile_pool(name="featp", bufs=2, space="PSUM") as fpool, \
                 tc.tile_pool(name="tanhp", bufs=3) as hpool:
                for b in range(BC):
                    # fp32 -> bf16 cast inside the DMA (SWDGE); quarter
                    # chunks so the first PE transposes can start early
                    for q in range(4):
                        cq = slice(q * (LC // 4), (q + 1) * (LC // 4))
                        lr = slice(q * (L // 4), (q + 1) * (L // 4))
                        nc.gpsimd.dma_start(
                            enc_bf[b][:, cq, :],
                            enc[b, lr].rearrange("(lc p) e -> p lc e", p=128))

                    # enc^T via PE transposes (LDWEIGHTS-bound, ~110ns/tile),
                    # evicted PSUM->SBUF by DVE in [128, 512] chunks
                    if FP8:
                        # [ki, drc, i, l]: e = drc*256 + i*128 + ki
                        encT = tpool.tile([128, 2, 2, L], fp8, name="encT",
                                          tag="encT")
                    else:
                        encT = tpool.tile([128, ECH, L], bf16, name="encT",
                                          tag="encT")
                    for lg in range(LG):
                        for ec in range(ECH):
                            pt = tpsp.tile([128, 4, 128], bf16, name="pt",
                                           tag="pt")
                            for j in range(4):
                                lc = lg * 4 + j
                                nc.tensor.transpose(
                                    pt[:, j, :],
                                    enc_bf[b][:, lc, ec * 128:(ec + 1) * 128],
                                    idT_sb[:])
                            if FP8:
                                dst = encT[:, ec // 2, ec % 2,
                                           lg * 512:(lg + 1) * 512]
                            else:
                                dst = encT[:, ec, lg * 512:(lg + 1) * 512]
                            nc.vector.tensor_copy(dst, pt[:])

                    for dc in range(DCH):
                        for lg in range(LG):
                            pf = fpool.tile([128, 512], f32, name="pf",
                                            tag="pf")
                            if FP8:
                                for drc in range(2):
                                    nc.tensor.matmul(
                                        pf[:],
                                        whT_sb[:, drc, :, dc, :],
                                        encT[:, drc, :,
                                             lg * 512:(lg + 1) * 512],
                                        start=(drc == 0), stop=False,
                                        perf_mode=DR)
                            else:
                                for ec in range(ECH):
                                    nc.tensor.matmul(
                                        pf[:],
                                        whT_sb[:, ec, dc * 128:(dc + 1) * 128],
                                        encT[:, ec, lg * 512:(lg + 1) * 512],
                                        start=(ec == 0), stop=False)
                            nc.tensor.matmul(
                                pf[:],
                                wce_sb[:, dc, b, :],
                                covb_sb[:, lg * 512:(lg + 1) * 512],
                                start=False, stop=True)
                            th = hpool.tile([128, 512], bf16, name="th",
                                            tag="th")
                            nc.scalar.activation(
                                th[:], pf[:], AF.Tanh,
                                bias=dec_sb[:, dc, b:b + 1],
                                scale=(1.0 / W_SCALE) if FP8 else 1.0)
                            nc.tensor.matmul(
                                psumS[:, lg, :],
                                ve_sb[:, dc, b, :],
                                th[:],
                                start=(b == 0 and dc == 0),
                                stop=(b == BC - 1 and dc == DCH - 1),
                                skip_group_check=True)

            # ---- softmax over l (4 batches on 4 partition lanes) -----------
            expS = cpool.tile([BC, L], f32)
            attnU = cpool.tile([BC, L], f32)
            sums = cpool.tile([BC, 1], f32)
            rsum = cpool.tile([BC, 1], f32)
            attnN = cpool.tile([BC, L], f32)
            nc.scalar.activation(expS[:], psumS[:], AF.Exp)
            sm1 = nc.vector.tensor_tensor(attnU[:], expS[:], maskf_sb[:],
                                          ALU.mult)
            sm2 = nc.vector.reduce_sum(sums[:], attnU[:], axis=AX.X)
            sm3 = nc.vector.reciprocal(rsum[:], sums[:])
            sm4 = nc.vector.tensor_scalar(attnN[:], attnU[:], rsum[:, 0:1],
                                          None, ALU.mult)
            nc.sync.dma_start(attn_out, attnN[:])
            # HAM warm-keepers: spread dummy matmuls across the softmax DVE
            # chain so the PE clock-gate stays at 8/8 for the context matmuls
            with tc.tile_pool(name="warmp", bufs=1, space="PSUM") as wpool:
                wp = wpool.tile([128, 128], f32)
                for dep in (sm1, sm2, sm3, sm4):
                    w = nc.tensor.matmul(wp[:], idT_sb[:], idT_sb[:],
                                         start=True, stop=True)
                    add_dep_helper(w.ins, dep.ins, reason="warmkeep spread")

            with tc.tile_pool(name="ctxp", bufs=1, space="PSUM") as xpool, \
                 tc.tile_pool(name="ctxsb", bufs=1) as xsb:
                # attn^T via PE transposes: [4, 128] slices -> [128, 4] PSUM
                attnT_ps = xpool.tile([128, LC, BC], f32)
                for lc in range(LC):
                    nc.tensor.transpose(attnT_ps[:, lc, :],
                                        attnN[:, lc * 128:(lc + 1) * 128],
                                        id4_sb[:])
                nc.vector.tensor_copy(attnT_sb[:], attnT_ps[:])

                # ---- context = attn @ enc ----------------------------------
                for b in range(BC):
                    pc = xpool.tile([1, E], f32, name="pc", tag="pc", bufs=2)
                    for lc in range(LC):
                        nc.tensor.matmul(
                            pc[:], attnT_sb[:, lc, b:b + 1],
                            enc_bf[b][:, lc, :],
                            start=(lc == 0), stop=(lc == LC - 1))
                    cr = xsb.tile([1, E], f32, name="cr", tag="cr", bufs=2)
                    nc.vector.tensor_copy(cr[:], pc[:])
                    nc.sync.dma_start(ctx_out[b:b + 1, :], cr[:])

    nc.compile()
    return nc


def _prep_inputs(enc_outputs, dec_state, coverage, mask, W_h, W_s, b_s, W_c, v):
    """Host-side prep: per-core sharding + (tiny) weight relayouts."""
    bf16 = ml_dtypes.bfloat16
    enc_outputs = np.ascontiguousarray(enc_outputs, dtype=np.float32)
    coverage = np.ascontiguousarray(coverage, dtype=np.float32)
    mask = np.ascontiguousarray(mask, dtype=np.int32)

    if FP8:
        # [ki, drc, i, dc, m]; e = drc*256 + i*128 + ki; scaled by W_SCALE
        fp8t = ml_dtypes.float8_e4m3
        w = (W_h.T.astype(np.float32) * W_SCALE).astype(fp8t)  # [e, d]
        whT = np.ascontiguousarray(
            w.reshape(2, 2, 128, DCH, 128)        # [drc, i, ki, dc, m]
            .transpose(2, 0, 1, 3, 4))            # [ki, drc, i, dc, m]
    else:
        # W_h^T [e, d] -> [p, ec, d]
        whT = np.ascontiguousarray(
            W_h.T.astype(bf16).reshape(ECH, 128, D).transpose(1, 0, 2))
    # W_s^T [k, d] -> [p, kc, d]
    wsT = np.ascontiguousarray(
        W_s.T.astype(bf16).reshape(DCH, 128, D).transpose(1, 0, 2))
    # b_s -> [1, dc, p] (folded into the dec matmul via a ones rhs)
    bsr = np.ascontiguousarray(b_s.astype(bf16).reshape(1, DCH, 128))
    id4 = np.eye(BC, dtype=np.float32)
    idT = np.eye(128, dtype=bf16)
    # v embedded per (dc, target batch-column)
    ve = np.zeros((128, DCH, BC, BC), dtype=bf16)
    vb = v[0].astype(bf16).reshape(DCH, 128)
    for dc in range(DCH):
        for b in range(BC):
            ve[:, dc, b, b] = vb[dc]
    # W_c embedded per (dc, batch-row); scaled like W_h in fp8 mode since the
    # whole features PSUM is descaled by the tanh activation's scale operand
    wce = np.zeros((BC, DCH, BC, 128), dtype=bf16)
    wcs = W_c[:, 0].astype(np.float32) * (W_SCALE if FP8 else 1.0)
    wcb = wcs.astype(bf16).reshape(DCH, 128)
    for dc in range(DCH):
        for b in range(BC):
            wce[b, dc, b, :] = wcb[dc]

    in_maps = []
    for c in range(NCORES):
        sl = slice(c * BC, (c + 1) * BC)
        dsT = np.ascontiguousarray(
            dec_state[sl].T.astype(bf16).reshape(DCH, 128, BC)
            .transpose(1, 0, 2))
        in_maps.append({
            "enc": enc_outputs[sl],
            "cov": coverage[sl],
            "maski": mask[sl],
            "whT": whT, "wsT": wsT, "dsT": dsT, "ve": ve, "wce": wce,
            "bsr": bsr, "id4": id4, "idT": idT,
        })
    return in_maps


def kernel(enc_outputs, dec_state, coverage, mask, W_h, W_s, b_s, W_c, v):
    global _PROG, LAST_RESULT
    from concourse.bass_utils import run_bass_kernel_spmd

    enc_outputs = np.asarray(enc_outputs)
    dec_state = np.asarray(dec_state)
    coverage = np.asarray(coverage)
    mask = np.asarray(mask)
    W_h = np.asarray(W_h)
    W_s = np.asarray(W_s)
    b_s = np.asarray(b_s)
    W_c = np.asarray(W_c)
    v = np.asarray(v)

    if _PROG is None:
        _PROG = _build_program()
    nc = _PROG

    in_maps = _prep_inputs(enc_outputs, dec_state, coverage, mask,
                           W_h, W_s, b_s, W_c, v)
    res = run_bass_kernel_spmd(nc, in_maps, core_ids=list(range(NCORES)),
                               trace=TRACE)
    LAST_RESULT = res

    context = np.empty((B, E), dtype=np.float32)
    attn = np.empty((B, L), dtype=np.float32)
    for c in range(NCORES):
        sl = slice(c * BC, (c + 1) * BC)
        context[sl] = res.results[c]["ctx_out"]
        attn[sl] = res.results[c]["attn_out"]
    return context, attn
